# revision 1
# baseline (speedup 1.0000x reference)
"""BiLSTM tagger (B=32, S=256, E=H=512, V=50000, T=64) on 8 Trainium2 cores.

Strategy (single SPMD launch):
  - Cores 0-3 run the forward direction, cores 4-7 the backward direction
    (clones within each group). The program is identical on all cores;
    direction comes only from the per-core input data.
  - The embedding gather happens on host (pure data movement); the device
    gets xs^T pre-packed in bf16.
  - Per layer: the x-projection (gates = Wx.T @ x, bias folded in via an
    augmented ones-row) runs as a large bf16 GEMM (matmul_tile_kernel);
    the sequential LSTM recurrence runs as a 256-step orientation-B scan
    (weights stationary per step, batch streamed, N=32 matmuls).
  - The two directions exchange h-sequences with pairwise AllGathers.
    Time-reversal of the peer sequence is never materialized: the next
    projection is split into an "own" GEMM (time-aligned) and a "peer" GEMM
    over the raw AllGather buffer whose non-peer rows are zeroed in the
    host-packed weights; the scan consumes that second stream with reversed
    block/step indexing.
  - Tag projection: same own/peer split; the small [64, S*B] peer result is
    reversed with a negative-stride DMA and added on-device.

Numerics: matmuls in bf16 with fp32 PSUM accumulation; gate activations,
cell state and elementwise updates in fp32; h stored bf16.
"""

import numpy as np
import ml_dtypes
import concourse.bass as bass
import concourse.bacc as bacc
import concourse.mybir as mybir
from concourse.tile import TileContext
from concourse.kernels.tile_matmul import matmul_tile_kernel
from concourse.bass_utils import run_bass_kernel_spmd

F32 = mybir.dt.float32
BF16 = mybir.dt.bfloat16
AF = mybir.ActivationFunctionType

S, B, E, H, V, T = 256, 32, 512, 512, 50000, 64
BLK = 8
REPLICA_GROUPS = [[0, 4], [1, 5], [2, 6], [3, 7]]
GATE_PERM = [0, 1, 3, 2]  # reference gate order [f,i,c,o] -> ours [f,i,o,ch]

ts = lambda i, n: slice(i * n, (i + 1) * n)


def _scan_cell(nc, tc, pools, wh_in, gx_ap, hout_ap, gxb_ap=None):
    """LSTM scan, orientation-B: psum bank [128,512] holds the 4 gate blocks
    [f|i|o|ch] in "chunked" layout (elem (p, 32a+b) = gate-dim 128a+p, batch b).
    gx_ap [2048, S*32] f32 is the precomputed x-projection (+bias), consumed
    time-aligned; gxb_ap is an optional second stream consumed time-reversed."""
    wpool, gxpool, state, gbuf, hring, psum = pools
    nb = S // BLK
    wt = []
    for k in range(4):
        w = wpool.tile([128, 2048], BF16, tag=f"wt{k}")
        nc.sync.dma_start(out=w[:, :], in_=wh_in[128 * k:128 * (k + 1), :])
        wt.append(w)
    h = state.tile([128, 128], BF16, tag="hst")
    c = state.tile([128, 128], F32, tag="cst")
    nc.vector.memset(h[:, :], 0.0)
    nc.vector.memset(c[:, :], 0.0)
    gsrc = gx_ap.rearrange("(m p) (s b) -> m p s b", p=128, b=32)
    gsrcb = gxb_ap.rearrange("(m p) (s b) -> m p s b", p=128, b=32) if gxb_ap is not None else None
    hdst = hout_ap.rearrange("(k p) (s b) -> k p s b", p=128, b=32)
    for blk in range(nb):
        # m-major block layout [p, (m, u, b)]: (u,b) contiguous on both sides
        gxt = gxpool.tile([128, BLK * 512], F32, tag="gx")
        dst = gxt[:, :].rearrange("p (m u b) -> p m u b", m=16, b=32)
        nc.sync.dma_start(out=dst, in_=gsrc[:, :, ts(blk, BLK), :].rearrange("m p u b -> p m u b"))
        if gsrcb is not None:
            gxtb = gxpool.tile([128, BLK * 512], F32, tag="gxb")
            dstb = gxtb[:, :].rearrange("p (m u b) -> p m u b", m=16, b=32)
            nc.sync.dma_start(out=dstb, in_=gsrcb[:, :, ts(nb - 1 - blk, BLK), :].rearrange("m p u b -> p m u b"))
        hb = hring.tile([128, BLK * 128], BF16, tag="hb")
        for u in range(BLK):
            ps = psum.tile([128, 512], F32, tag="ps")
            for m in range(16):
                for k in range(4):
                    nc.tensor.matmul(
                        ps[:, 32 * m:32 * m + 32],
                        lhsT=wt[k][:, 128 * m:128 * (m + 1)],
                        rhs=h[:, 32 * k:32 * k + 32],
                        start=(k == 0), stop=(k == 3),
                    )
            g = gbuf.tile([128, 512], F32, tag="g")
            gx4 = gxt[:, :].rearrange("p (m u b) -> p m u b", m=16, b=32)
            g4 = g[:, :].rearrange("p (m b) -> p m b", b=32)
            ps4 = ps[:, :].rearrange("p (m b) -> p m b", b=32)
            nc.vector.tensor_add(out=g4, in0=ps4, in1=gx4[:, :, u, :])
            if gsrcb is not None:
                gxb4 = gxtb[:, :].rearrange("p (m u b) -> p m u b", m=16, b=32)
                nc.vector.tensor_add(out=g4, in0=g4, in1=gxb4[:, :, BLK - 1 - u, :])
            act = gbuf.tile([128, 512], F32, tag="act")
            nc.scalar.activation(act[:, 0:384], g[:, 0:384], AF.Sigmoid)
            nc.scalar.activation(act[:, 384:512], g[:, 384:512], AF.Tanh)
            t1 = gbuf.tile([128, 128], F32, tag="t1")
            t2 = gbuf.tile([128, 128], F32, tag="t2")
            nc.vector.tensor_mul(out=t1[:, :], in0=act[:, 0:128], in1=c[:, :])
            nc.vector.tensor_mul(out=t2[:, :], in0=act[:, 128:256], in1=act[:, 384:512])
            nc.vector.tensor_add(out=c[:, :], in0=t1[:, :], in1=t2[:, :])
            nc.scalar.activation(t1[:, :], c[:, :], AF.Tanh)
            nc.vector.tensor_mul(out=h[:, :], in0=act[:, 256:384], in1=t1[:, :])
            nc.vector.tensor_copy(out=hb[:, u * 128:(u + 1) * 128], in_=h[:, :])
        hsrc = hb[:, :].rearrange("p (u k b) -> k p u b", k=4, b=32)
        for k in range(4):
            nc.sync.dma_start(out=hdst[k, :, ts(blk, BLK), :], in_=hsrc[k, :, :, :])


def _build_full():
    N = S * 32
    nc = bacc.Bacc("TRN2", target_bir_lowering=False, debug=False, num_devices=8)
    xsT = nc.dram_tensor("xsT", [640, N], BF16, kind="ExternalInput")
    wx1 = nc.dram_tensor("wx1", [640, 2048], BF16, kind="ExternalInput")
    wh1 = nc.dram_tensor("wh1", [512, 2048], BF16, kind="ExternalInput")
    wx2a = nc.dram_tensor("wx2a", [640, 2048], BF16, kind="ExternalInput")
    wx2b = nc.dram_tensor("wx2b", [1024, 2048], BF16, kind="ExternalInput")
    wh2 = nc.dram_tensor("wh2", [512, 2048], BF16, kind="ExternalInput")
    wtaga = nc.dram_tensor("wtaga", [512, 64], BF16, kind="ExternalInput")
    wtagb = nc.dram_tensor("wtagb", [1024, 64], BF16, kind="ExternalInput")
    tags = nc.dram_tensor("tags", [64, N], F32, kind="ExternalOutput")

    with TileContext(nc) as tc:
        with (
            tc.tile_pool(name="dram", bufs=1, space="DRAM") as dram,
            tc.tile_pool(name="cons", bufs=1) as cons,
            tc.tile_pool(name="wpool", bufs=1) as wpool,
            tc.tile_pool(name="gxpool", bufs=2) as gxpool,
            tc.tile_pool(name="state", bufs=1) as state,
            tc.tile_pool(name="gbuf", bufs=2) as gbuf,
            tc.tile_pool(name="hring", bufs=2) as hring,
            tc.tile_pool(name="psum", bufs=2, space="PSUM") as psum,
        ):
            pools = (wpool, gxpool, state, gbuf, hring, psum)

            gx1 = dram.tile([2048, N], F32, tag="gx1")
            matmul_tile_kernel(tc, wx1[:, :], xsT[:, :], gx1[:, :])
            hout1 = dram.tile([512, N], BF16, tag="hout1")
            _scan_cell(nc, tc, pools, wh1, gx1[:, :], hout1[:, :])

            ag1 = dram.tile([1024, N], BF16, tag="ag1")
            nc.gpsimd.collective_compute(
                "AllGather", mybir.AluOpType.bypass,
                replica_groups=REPLICA_GROUPS,
                ins=[hout1.opt()], outs=[ag1.opt()],
            )
            catA = dram.tile([640, N], BF16, tag="catA")
            nc.sync.dma_start(out=catA[0:512, :], in_=hout1[:, :])
            ones = cons.tile([128, N], BF16, tag="aug")
            nc.vector.memset(ones[:, :], 0.0)
            nc.vector.memset(ones[0:1, :], 1.0)
            nc.sync.dma_start(out=catA[512:640, :], in_=ones[:, :])

            gx2a = dram.tile([2048, N], F32, tag="gx2a")
            matmul_tile_kernel(tc, wx2a[:, :], catA[:, :], gx2a[:, :])
            gx2b = dram.tile([2048, N], F32, tag="gx2b")
            matmul_tile_kernel(tc, wx2b[:, :], ag1[:, :], gx2b[:, :])

            hout2 = dram.tile([512, N], BF16, tag="hout2")
            _scan_cell(nc, tc, pools, wh2, gx2a[:, :], hout2[:, :], gxb_ap=gx2b[:, :])

            ag2 = dram.tile([1024, N], BF16, tag="ag2")
            nc.gpsimd.collective_compute(
                "AllGather", mybir.AluOpType.bypass,
                replica_groups=REPLICA_GROUPS,
                ins=[hout2.opt()], outs=[ag2.opt()],
            )
            tagA = dram.tile([64, N], F32, tag="tagA")
            matmul_tile_kernel(tc, wtaga[:, :], hout2[:, :], tagA[:, :])
            tagB = dram.tile([64, N], F32, tag="tagB")
            matmul_tile_kernel(tc, wtagb[:, :], ag2[:, :], tagB[:, :])
            CH = min(2048, N)
            for j in range(N // CH):
                ta = cons.tile([64, CH], F32, tag="ta")
                tb = cons.tile([64, CH], F32, tag="tb")
                nc.sync.dma_start(out=ta[:, :], in_=tagA[:, ts(j, CH)])
                nbk = CH // 32
                sbv = tagB[:, :].rearrange("r (s b) -> r s b", b=32)
                src = sbv[:, ts(N // CH - 1 - j, nbk), :]
                rsrc = bass.AP(src.tensor, src.offset + (nbk - 1) * 32,
                               [[N, 64], [-32, nbk], [1, 32]])
                nc.sync.dma_start(out=tb[:, :], in_=rsrc)
                nc.vector.tensor_add(out=ta[:, :], in0=ta[:, :], in1=tb[:, :])
                nc.sync.dma_start(out=tags[:, ts(j, CH)], in_=ta[:, :])
    nc.compile()
    return nc


def _bf(x):
    return np.ascontiguousarray(x).astype(ml_dtypes.bfloat16)


def _pack_inputs(words, emb, Wf1, bf1, Wb1, bb1, Wf2, bf2, Wb2, bb2, Wtag):
    words = np.asarray(words).astype(np.int64)
    xs = np.asarray(emb, dtype=np.float32)[words]      # [B, S, E] host gather
    xsT_f = xs.transpose(2, 1, 0).reshape(512, S * B)  # col = t*B + b
    xsT_b = xs[:, ::-1].transpose(2, 1, 0).reshape(512, S * B)

    def pack_xs(xsT):
        out = np.zeros((640, xsT.shape[1]), np.float32)
        out[:512] = xsT
        out[512] = 1.0
        return _bf(out)

    def gates_cat(W, rows):
        return np.concatenate([np.asarray(W[g], dtype=np.float32)[rows]
                               for g in GATE_PERM], axis=1)

    def bias_cat(b):
        return np.concatenate([np.asarray(b[g], dtype=np.float32) for g in GATE_PERM])

    def pack_dir(Wl1, bl1, Wl2, bl2, fwd):
        wx1p = np.zeros((640, 2048), np.float32)
        wx1p[:512] = gates_cat(Wl1, slice(0, 512))
        wx1p[512] = bias_cat(bl1)
        wh1p = gates_cat(Wl1, slice(512, 1024))
        ownsl = slice(0, 512) if fwd else slice(512, 1024)
        wx2m = gates_cat(Wl2, slice(0, 1024))
        wx2ap = np.zeros((640, 2048), np.float32)
        wx2ap[:512] = wx2m[ownsl]
        wx2ap[512] = bias_cat(bl2)
        wx2bp = wx2m.copy()
        wx2bp[ownsl] = 0.0
        wh2p = gates_cat(Wl2, slice(1024, 1536))
        wt = np.asarray(Wtag, dtype=np.float32)
        wtagap = wt[ownsl]
        wtagbp = wt.copy()
        wtagbp[ownsl] = 0.0
        return dict(
            xsT=pack_xs(xsT_f if fwd else xsT_b),
            wx1=_bf(wx1p), wh1=_bf(wh1p),
            wx2a=_bf(wx2ap), wx2b=_bf(wx2bp), wh2=_bf(wh2p),
            wtaga=_bf(wtagap), wtagb=_bf(wtagbp),
        )

    fw = pack_dir(Wf1, bf1, Wf2, bf2, True)
    bw = pack_dir(Wb1, bb1, Wb2, bb2, False)
    return [fw] * 4 + [bw] * 4


_NC_CACHE = {}


def _get_nc():
    if "nc" not in _NC_CACHE:
        _NC_CACHE["nc"] = _build_full()
    return _NC_CACHE["nc"]


def kernel(words, lengths, emb, Wf1, bf1, Wb1, bb1, Wf2, bf2, Wb2, bb2, Wtag, btag):
    nc = _get_nc()
    ins = _pack_inputs(words, emb, Wf1, bf1, Wb1, bb1, Wf2, bf2, Wb2, bb2, Wtag)
    res = run_bass_kernel_spmd(nc, ins, core_ids=list(range(8)))
    tags = res.results[0]["tags"]                       # [64, S*B], col = t*B+b
    out = tags.reshape(T, S, B).transpose(2, 1, 0).reshape(B * S, T)
    out = out + np.asarray(btag, dtype=np.float32)[None, :]
    return np.ascontiguousarray(out.astype(np.float32))



# revision 8
# speedup vs baseline: 1534.2541x; 1534.2541x over previous
"""BiLSTM tagger (B=32, S=256, E=H=512, V=50000, T=64) on 8 Trainium2 cores.

Strategy (single SPMD launch, 8-way real parallelism):
  - Core c in 0-3 runs the FORWARD direction for batch slice [8c, 8c+8);
    core c+4 runs the BACKWARD direction for the same slice (its xsT is
    host-reversed in time). All per-core tensors hold 8 batch elements ×
    256 steps = 2048 columns (col = t*8 + b).
  - Per layer: the x-projection (+bias via an augmented ones-row) is a
    small bf16 GEMM; the sequential LSTM recurrence is a 256-step scan
    with weights stationary on the PE (batch=8 moving) and a short
    elementwise chain (in-place psum adds, ch-gates first so the tanh
    path hides under the f/i/o matmuls, fused [f|i]*[c|ch] product, h
    consumed directly from the h ring).
  - Each scan writes its h sequence twice: time-aligned into the next
    GEMM's input buffer, and time-REVERSED into an exchange tensor. The
    pairwise AllGather (core c <-> c+4, same batch slice) of the reversed
    copy lands peer h already aligned to the local time axis, so layer 2
    and the tag projection are each a single GEMM over
    [own-h | allgathered | ones] with no reversed second stream.
  - The host assembles the full [B*S, T] output from the four forward
    cores' batch slices.

Numerics: matmuls bf16 with fp32 PSUM accumulation; gate activations and
cell state fp32; gx streams and h stored bf16.
"""

import numpy as np
import ml_dtypes
import concourse.bass as bass
import concourse.bacc as bacc
import concourse.mybir as mybir
from concourse.tile import TileContext
from concourse.kernels.tile_matmul import matmul_tile_kernel
from concourse.bass_utils import run_bass_kernel_spmd

F32 = mybir.dt.float32
BF16 = mybir.dt.bfloat16
AF = mybir.ActivationFunctionType

S, B, E, H, V, T = 256, 32, 512, 512, 50000, 64
BS = 8                  # batch elements per core
NC = S * BS             # 2048 columns per core
BLK = 8                 # steps per block
NB = S // BLK           # 32 blocks
CH = 4                  # column chunks (tensors) for gx/exchange buffers
CW = NC // CH           # 512 cols per chunk
BPC = NB // CH          # 8 blocks per chunk
GRP = BLK * BS          # 64 cols per block
REPLICA_GROUPS = [[0, 4], [1, 5], [2, 6], [3, 7]]
GATE_PERM = [0, 1, 3, 2]  # reference gate order [f,i,c,o] -> ours [f,i,o,ch]

ts = lambda i, n: slice(i * n, (i + 1) * n)


def _scan_cell(nc, tc, pools, wh_in, gx_chunks, own_chunks, rev_chunks):
    """LSTM scan. psA [128, 96] holds [f|i|o] gates, psB [128, 32] holds ch
    (elem (p, 8m+b) = gate-dim 128m+p within the block, batch b). ch matmuls
    are emitted first so its tanh overlaps the f/i/o matmuls. gx_chunks: CH
    dram tensors [2048, CW] bf16 (x-projection+bias), consumed time-aligned.
    own_chunks/rev_chunks: per-chunk dram views receiving h time-aligned /
    time-reversed."""
    wpool, gxpool, state, work, hring, psum = pools
    wt = []
    for k in range(4):
        w = wpool.tile([128, 2048], BF16, tag=f"wt{k}")
        nc.sync.dma_start(out=w[:, :], in_=wh_in[128 * k:128 * (k + 1), :])
        wt.append(w)
    cch = state.tile([128, 2 * BS * 4], F32, tag="cch")   # [c | ch]
    hz = state.tile([128, BS * 4], BF16, tag="hz")
    nc.vector.memset(cch[:, :], 0.0)
    nc.vector.memset(hz[:, :], 0.0)
    CB = BS * 4  # 32: one h/c slot width

    def fetch(blk):
        gsrc = gx_chunks[blk // BPC].rearrange("(m p) (s b) -> m p s b",
                                               p=128, b=BS)
        gxt = gxpool.tile([128, BLK * 16 * BS], BF16, tag="gx")
        dst = gxt[:, :].rearrange("p (m u b) -> p m u b", m=16, b=BS)
        nc.sync.dma_start(
            out=dst,
            in_=gsrc[:, :, ts(blk % BPC, BLK), :].rearrange("m p u b -> p m u b"))
        return gxt

    prev_hb = None
    gxt = fetch(0)
    for blk in range(NB):
        lblk = blk % BPC
        gxt_next = fetch(blk + 1) if blk + 1 < NB else None
        g4 = gxt[:, :].rearrange("p (m u b) -> p m u b", m=16, b=BS)
        hb = hring.tile([128, BLK * CB], BF16, tag="hb")
        for u in range(BLK):
            if u == 0:
                hsrc_prev = hz if prev_hb is None else prev_hb
                hoff = 0 if prev_hb is None else (BLK - 1) * CB
            else:
                hsrc_prev = hb
                hoff = (u - 1) * CB
            psA = psum.tile([128, 12 * BS], F32, tag="psA")
            psB = psum.tile([128, 4 * BS], F32, tag="psB")
            for m in range(12, 16):
                for k in range(4):
                    nc.tensor.matmul(
                        psB[:, ts(m - 12, BS)],
                        lhsT=wt[k][:, 128 * m:128 * (m + 1)],
                        rhs=hsrc_prev[:, hoff + BS * k:hoff + BS * k + BS],
                        start=(k == 0), stop=(k == 3),
                    )
            for m in range(12):
                for k in range(4):
                    nc.tensor.matmul(
                        psA[:, ts(m, BS)],
                        lhsT=wt[k][:, 128 * m:128 * (m + 1)],
                        rhs=hsrc_prev[:, hoff + BS * k:hoff + BS * k + BS],
                        start=(k == 0), stop=(k == 3),
                    )
            nc.vector.tensor_add(
                out=psB[:, :].rearrange("p (m b) -> p m b", b=BS),
                in0=psB[:, :].rearrange("p (m b) -> p m b", b=BS),
                in1=g4[:, 12:16, u])
            nc.scalar.activation(cch[:, CB:2 * CB], psB[:, :], AF.Tanh)
            nc.vector.tensor_add(
                out=psA[:, :].rearrange("p (m b) -> p m b", b=BS),
                in0=psA[:, :].rearrange("p (m b) -> p m b", b=BS),
                in1=g4[:, 0:12, u])
            act = work.tile([128, 12 * BS], F32, tag="act")  # [f|i|o]
            nc.scalar.activation(act[:, :], psA[:, :], AF.Sigmoid)
            P = work.tile([128, 2 * CB], F32, tag="P")
            nc.vector.tensor_mul(out=P[:, :], in0=act[:, 0:2 * CB], in1=cch[:, :])
            nc.vector.tensor_add(out=cch[:, 0:CB], in0=P[:, 0:CB],
                                 in1=P[:, CB:2 * CB])
            th = work.tile([128, CB], F32, tag="th")
            nc.scalar.activation(th[:, :], cch[:, 0:CB], AF.Tanh)
            nc.vector.tensor_mul(out=hb[:, u * CB:(u + 1) * CB],
                                 in0=act[:, 2 * CB:3 * CB], in1=th[:, :])
        # writebacks: own (time-aligned) + reversed copy for the exchange;
        # issued on the scalar queue to keep the SP queue free for gx loads
        hsrc = hb[:, :].rearrange("p (u k b) -> k p u b", k=4, b=BS)
        own = own_chunks[blk // BPC].rearrange("(k p) (s b) -> k p s b",
                                               p=128, b=BS)
        rc = NB - 1 - blk
        rev = rev_chunks[rc // BPC].rearrange("(k p) (s b) -> k p s b",
                                              p=128, b=BS)
        for k in range(4):
            nc.scalar.dma_start(out=own[k, :, ts(lblk, BLK), :],
                                in_=hsrc[k, :, :, :])
            dsl = rev[k, :, ts(rc % BPC, BLK), :]
            rdst = bass.AP(dsl.tensor, dsl.offset + (BLK - 1) * BS,
                           [[CW, 128], [-BS, BLK], [1, BS]])
            nc.scalar.dma_start(out=rdst, in_=hsrc[k, :, :, :])
        prev_hb = hb
        gxt = gxt_next


def _build_full(repeat=1, sim_single_core=False):
    """repeat>1 loops the whole body (slope-based HW timing: wall ~=
    dispatch + repeat*hw). sim_single_core replaces the AllGathers with
    local DRAM copies so TimelineSim (single-core, no collectives) can
    profile the program."""
    nc = bacc.Bacc("TRN2", target_bir_lowering=False, debug=False, num_devices=8)
    xsT = nc.dram_tensor("xsT", [640, NC], BF16, kind="ExternalInput")
    wx1 = nc.dram_tensor("wx1", [640, 2048], BF16, kind="ExternalInput")
    wh1 = nc.dram_tensor("wh1", [512, 2048], BF16, kind="ExternalInput")
    wx2 = nc.dram_tensor("wx2", [1664, 2048], BF16, kind="ExternalInput")
    wh2 = nc.dram_tensor("wh2", [512, 2048], BF16, kind="ExternalInput")
    wtag = nc.dram_tensor("wtag", [1664, 64], BF16, kind="ExternalInput")
    tags = nc.dram_tensor("tags", [64, NC], F32, kind="ExternalOutput")

    def allgather(srcs, dsts):
        # dsts[c] is rows 512:1536 of the [1664, CW] input chunk
        for c in range(CH - 1, -1, -1):
            if sim_single_core:
                nc.sync.dma_start(out=dsts[c][0:512, :], in_=srcs[c][:, :])
                nc.sync.dma_start(out=dsts[c][512:1024, :], in_=srcs[c][:, :])
            else:
                nc.gpsimd.collective_compute(
                    "AllGather", mybir.AluOpType.bypass,
                    replica_groups=REPLICA_GROUPS,
                    ins=[srcs[c][:, :]], outs=[dsts[c][:, :]],
                )

    with TileContext(nc) as tc:
        with (
            tc.tile_pool(name="dram", bufs=1, space="DRAM") as dram,
            tc.tile_pool(name="cons", bufs=1) as cons,
            tc.tile_pool(name="wpool", bufs=1) as wpool,
            tc.tile_pool(name="gxpool", bufs=2) as gxpool,
            tc.tile_pool(name="state", bufs=1) as state,
            tc.tile_pool(name="work", bufs=2) as work,
            tc.tile_pool(name="hring", bufs=2) as hring,
            tc.tile_pool(name="psum", bufs=2, space="PSUM") as psum,
        ):
          pools = (wpool, gxpool, state, work, hring, psum)

          def dtile(shape, tag):
              return dram.tile(shape, BF16, tag=tag, name=tag)

          gx1 = [dtile([2048, CW], f"gx1_{c}") for c in range(CH)]
          gx2 = [dtile([2048, CW], f"gx2_{c}") for c in range(CH)]
          g2in = [dtile([1664, CW], f"g2in_{c}") for c in range(CH)]
          tgin = [dtile([1664, CW], f"tgin_{c}") for c in range(CH)]
          h1r = [dtile([512, CW], f"h1r_{c}") for c in range(CH)]
          h2r = [dtile([512, CW], f"h2r_{c}") for c in range(CH)]

          def gemm(kxm, kxn, out):
              matmul_tile_kernel(tc, kxm, kxn, out)
              tc.swap_default_side()  # undo the kernel's side toggle

          for _rep in range(repeat):
            if _rep:
                # Serialize iterations for slope timing: stage a casted copy
                # of the previous iteration's output into the gx1 chunk the
                # first GEMM overwrites, so iteration r+1 cannot start early.
                ser = cons.tile([64, CW], F32, tag="ser")
                serb = cons.tile([64, CW], BF16, tag="serb")
                nc.sync.dma_start(out=ser[:, :], in_=tags[:, 0:CW])
                nc.vector.tensor_copy(out=serb[:, :], in_=ser[:, :])
                nc.sync.dma_start(out=gx1[0][0:64, :], in_=serb[:, :])

            ones = cons.tile([128, CW], BF16, tag="aug")
            nc.vector.memset(ones[:, :], 0.0)
            nc.vector.memset(ones[0:1, :], 1.0)
            for c in range(CH):
                nc.sync.dma_start(out=g2in[c][1536:1664, :], in_=ones[:, :])
                nc.sync.dma_start(out=tgin[c][1536:1664, :], in_=ones[:, :])

            for c in range(CH):
                gemm(wx1[:, :], xsT[:, ts(c, CW)], gx1[c][:, :])
            _scan_cell(nc, tc, pools, wh1, gx1,
                       [t[0:512, :] for t in g2in], h1r)
            allgather(h1r, [t[512:1536, :] for t in g2in])

            for c in range(CH):
                gemm(wx2[:, :], g2in[c][:, :], gx2[c][:, :])
            _scan_cell(nc, tc, pools, wh2, gx2,
                       [t[0:512, :] for t in tgin], h2r)
            allgather(h2r, [t[512:1536, :] for t in tgin])

            for c in range(CH):
                gemm(wtag[:, :], tgin[c][:, :], tags[:, ts(c, CW)])
    nc.compile()
    return nc


def _bf(x):
    return np.ascontiguousarray(x).astype(ml_dtypes.bfloat16)


def _pack_inputs(words, emb, Wf1, bf1, Wb1, bb1, Wf2, bf2, Wb2, bb2, Wtag, btag):
    words = np.asarray(words).astype(np.int64)
    xs = np.asarray(emb, dtype=np.float32)[words]      # [B, S, E] host gather

    def pack_xs(xs_slice):
        out = np.zeros((640, NC), np.float32)
        out[:512] = xs_slice.transpose(2, 1, 0).reshape(512, NC)
        out[512] = 1.0
        return _bf(out)

    def gates_cat(W, rows):
        return np.concatenate([np.asarray(W[g], dtype=np.float32)[rows]
                               for g in GATE_PERM], axis=1)

    def bias_cat(b):
        return np.concatenate([np.asarray(b[g], dtype=np.float32) for g in GATE_PERM])

    wt = np.asarray(Wtag, dtype=np.float32)
    bt = np.asarray(btag, dtype=np.float32)

    def pack_dir(Wl1, bl1, Wl2, bl2, fwd):
        wx1p = np.zeros((640, 2048), np.float32)
        wx1p[:512] = gates_cat(Wl1, slice(0, 512))
        wx1p[512] = bias_cat(bl1)
        wh1p = gates_cat(Wl1, slice(512, 1024))
        w2x = gates_cat(Wl2, slice(0, 1024))   # rows 0:512 ~ h_f, 512:1024 ~ h_b
        own = slice(0, 512) if fwd else slice(512, 1024)
        peer = slice(512, 1024) if fwd else slice(0, 512)
        # allgathered rows: [0:512] = fwd contribution, [512:1024] = bwd
        peer_off = 1024 if fwd else 512
        wx2p = np.zeros((1664, 2048), np.float32)
        wx2p[0:512] = w2x[own]
        wx2p[peer_off:peer_off + 512] = w2x[peer]
        wx2p[1536] = bias_cat(bl2)
        wh2p = gates_cat(Wl2, slice(1024, 1536))
        wtagp = np.zeros((1664, 64), np.float32)
        wtagp[0:512] = wt[own]
        wtagp[peer_off:peer_off + 512] = wt[peer]
        wtagp[1536] = bt
        return dict(
            wx1=_bf(wx1p), wh1=_bf(wh1p),
            wx2=_bf(wx2p), wh2=_bf(wh2p),
            wtag=_bf(wtagp),
        )

    fw = pack_dir(Wf1, bf1, Wf2, bf2, True)
    bw = pack_dir(Wb1, bb1, Wb2, bb2, False)
    ins = []
    for core in range(8):
        sl = xs[ts(core % 4, BS)]
        d = dict(fw if core < 4 else bw)
        d["xsT"] = pack_xs(sl if core < 4 else sl[:, ::-1])
        ins.append(d)
    return ins


_NC_CACHE = {}


def _get_nc():
    if "nc" not in _NC_CACHE:
        _NC_CACHE["nc"] = _build_full()
    return _NC_CACHE["nc"]


def kernel(words, lengths, emb, Wf1, bf1, Wb1, bb1, Wf2, bf2, Wb2, bb2, Wtag, btag):
    nc = _get_nc()
    ins = _pack_inputs(words, emb, Wf1, bf1, Wb1, bb1, Wf2, bf2, Wb2, bb2,
                       Wtag, btag)
    res = run_bass_kernel_spmd(nc, ins, core_ids=list(range(8)))
    # core c (forward) holds tags for batch slice [8c, 8c+8), col = t*8+b
    full = np.concatenate(
        [res.results[c]["tags"].reshape(T, S, BS) for c in range(4)], axis=2)
    out = full.transpose(2, 1, 0).reshape(B * S, T)
    return np.ascontiguousarray(out.astype(np.float32))


# revision 30
# speedup vs baseline: 2005.8030x; 1.3073x over previous
"""BiLSTM tagger (B=32, S=256, E=H=512, V=50000, T=64) on 8 Trainium2 cores.

Strategy (single SPMD launch, 8-way real parallelism):
  - Core c in 0-3 runs the FORWARD direction for batch slice [8c, 8c+8);
    core c+4 runs the BACKWARD direction for the same slice (its xsT is
    host-reversed in time). All per-core tensors hold 8 batch elements ×
    256 steps = 2048 columns (col = t*8 + b).
  - Per layer: the x-projection (+bias via an augmented ones-row) is a
    small bf16 GEMM; the sequential LSTM recurrence is a 256-step scan
    with weights stationary on the PE (batch=8 moving) and a short
    elementwise chain (in-place psum adds, ch-gates first so the tanh
    path hides under the f/i/o matmuls, fused [f|i]*[c|ch] product, h
    consumed directly from the h ring).
  - Each scan writes its h sequence twice: time-aligned into the next
    GEMM's input buffer, and time-REVERSED into an exchange tensor. The
    pairwise AllGather (core c <-> c+4, same batch slice) of the reversed
    copy lands peer h already aligned to the local time axis, so layer 2
    and the tag projection are each a single GEMM over
    [own-h | allgathered | ones] with no reversed second stream.
  - The host assembles the full [B*S, T] output from the four forward
    cores' batch slices.

Numerics: matmuls bf16 with fp32 PSUM accumulation; gate activations and
cell state fp32; gx streams and h stored bf16.
"""

import os
import numpy as np
import ml_dtypes
import concourse.bass as bass
import concourse.bacc as bacc
import concourse.mybir as mybir
from concourse.tile import TileContext
from concourse.kernels.tile_matmul import matmul_tile_kernel
from concourse.bass_utils import run_bass_kernel_spmd

F32 = mybir.dt.float32
BF16 = mybir.dt.bfloat16
AF = mybir.ActivationFunctionType

S, B, E, H, V, T = 256, 32, 512, 512, 50000, 64
BS = int(os.environ.get("K_BS", "32"))   # batch elements per core
NC = S * BS             # 2048 columns per core
BLK = 8                 # steps per block
NB = S // BLK           # 32 blocks
CH = 4                  # column chunks (tensors) for gx/exchange buffers
CW = NC // CH           # 512 cols per chunk
BPC = NB // CH          # 8 blocks per chunk
GRP = BLK * BS          # 64 cols per block
REPLICA_GROUPS = [[0, 4], [1, 5], [2, 6], [3, 7]]
GATE_PERM = [0, 1, 3, 2]  # reference gate order [f,i,c,o] -> ours [f,i,o,ch]

ts = lambda i, n: slice(i * n, (i + 1) * n)


NH = int(os.environ.get("K_NH", "1"))   # pipeline sub-streams (batch halves)
HB = BS // NH                           # batch per sub-stream
INJ = int(os.environ.get("K_INJ", "0"))  # 1: inject gx via identity matmul;
                                         # 0: DVE adds into psum


def _scan_cell(nc, tc, pools, wh_in, gx_chunks, own_chunks, rev_chunks, ident):
    """LSTM scan, two interleaved batch sub-streams so one stream's
    elementwise chain overlaps the other's matmuls. Per (step, stream):
    psB [128, 4*HB] holds ch gates (emitted first; its tanh hides under the
    f/i/o matmuls into psA [128, 12*HB]); the x-projection gx is INJECTED
    into psum with an identity matmul (no DVE adds on the chain). gx_chunks:
    CH dram tensors [2048, CW] bf16, consumed time-aligned; own/rev_chunks
    receive h time-aligned / time-reversed."""
    wpool, gxpool, state, work, hring, (psum, psumB) = pools
    wt = []
    for k in range(4):
        w = wpool.tile([128, 2048], BF16, tag=f"wt{k}")
        nc.sync.dma_start(out=w[:, :], in_=wh_in[128 * k:128 * (k + 1), :])
        wt.append(w)
    CB = BS * 4       # one h slot width
    cch = [state.tile([128, 8 * HB], F32, tag=f"cch{g}", name=f"cch{g}")
           for g in range(NH)]          # [c | ch] per stream
    hz = state.tile([128, CB], BF16, tag="hz")
    for g in range(NH):
        nc.vector.memset(cch[g][:, :], 0.0)
    nc.vector.memset(hz[:, :], 0.0)

    def fetch(blk):
        # gxt cols = (m', u, g, b) with m' = [12..15, 0..11]: ch gates first
        gsrc = gx_chunks[blk // BPC].rearrange("(m p) (s b) -> m p s b",
                                               p=128, b=BS)
        gxt = gxpool.tile([128, BLK * 16 * BS], BF16, tag="gx")
        dst = gxt[:, :].rearrange("p (m u b) -> p m u b", m=16, b=BS)
        sl = gsrc[:, :, ts(blk % BPC, BLK), :]
        nc.sync.dma_start(out=dst[:, 0:4],
                          in_=sl[12:16].rearrange("m p u b -> p m u b"))
        nc.sync.dma_start(out=dst[:, 4:16],
                          in_=sl[0:12].rearrange("m p u b -> p m u b"))
        return gxt

    prev_hb = None
    gxt = fetch(0)
    for blk in range(NB):
        lblk = blk % BPC
        gxt_next = fetch(blk + 1) if blk + 1 < NB else None
        hb = hring.tile([128, BLK * CB], BF16, tag="hb")
        for u in range(BLK):
            if u == 0:
                hsrc_prev = hz if prev_hb is None else prev_hb
                hoff = 0 if prev_hb is None else (BLK - 1) * CB
            else:
                hsrc_prev = hb
                hoff = (u - 1) * CB
            g4 = gxt[:, :].rearrange("p (m u g b) -> p m u g b",
                                     m=16, u=BLK, g=NH)
            for g in range(NH):
                boff = HB * g
                psA = psum.tile([128, 12 * HB], F32, tag=f"psA{g}",
                                name=f"psA{g}")
                psB = psumB.tile([128, 4 * HB], F32, tag=f"psB{g}",
                                 name=f"psB{g}")
                for m in range(12, 16):
                    for k in range(4):
                        nc.tensor.matmul(
                            psB[:, ts(m - 12, HB)],
                            lhsT=wt[k][:, 128 * m:128 * (m + 1)],
                            rhs=hsrc_prev[:, hoff + BS * k + boff:
                                          hoff + BS * k + boff + HB],
                            start=(k == 0), stop=(not INJ and k == 3),
                        )
                if INJ:
                    nc.tensor.matmul(psB[:, :], lhsT=ident[:, :],
                                     rhs=g4[:, 0:4, u, g, :],
                                     start=False, stop=True)
                else:
                    nc.vector.tensor_add(
                        out=psB[:, :].rearrange("p (m b) -> p m b", b=HB),
                        in0=psB[:, :].rearrange("p (m b) -> p m b", b=HB),
                        in1=g4[:, 0:4, u, g, :])
                for m in range(12):
                    for k in range(4):
                        nc.tensor.matmul(
                            psA[:, ts(m, HB)],
                            lhsT=wt[k][:, 128 * m:128 * (m + 1)],
                            rhs=hsrc_prev[:, hoff + BS * k + boff:
                                          hoff + BS * k + boff + HB],
                            start=(k == 0), stop=(not INJ and k == 3),
                        )
                if INJ:
                    nc.tensor.matmul(psA[:, :], lhsT=ident[:, :],
                                     rhs=g4[:, 4:16, u, g, :],
                                     start=False, stop=True)
                else:
                    nc.vector.tensor_add(
                        out=psA[:, :].rearrange("p (m b) -> p m b", b=HB),
                        in0=psA[:, :].rearrange("p (m b) -> p m b", b=HB),
                        in1=g4[:, 4:16, u, g, :])
                nc.scalar.activation(cch[g][:, 4 * HB:8 * HB], psB[:, :],
                                     AF.Tanh)
                act = work.tile([128, 12 * HB], F32, tag=f"act{g}",
                                name=f"act{g}")  # [f|i|o]
                nc.scalar.activation(act[:, :], psA[:, :], AF.Sigmoid)
                P = work.tile([128, 8 * HB], F32, tag=f"P{g}", name=f"P{g}")
                nc.vector.tensor_mul(out=P[:, :], in0=act[:, 0:8 * HB],
                                     in1=cch[g][:, :])
                nc.vector.tensor_add(out=cch[g][:, 0:4 * HB], in0=P[:, 0:4 * HB],
                                     in1=P[:, 4 * HB:8 * HB])
                th = work.tile([128, 4 * HB], F32, tag=f"th{g}", name=f"th{g}")
                nc.scalar.activation(th[:, :], cch[g][:, 0:4 * HB], AF.Tanh)
                hdst = hb[:, :].rearrange("p (uu k b) -> p uu k b",
                                          uu=BLK, b=BS)[:, u, :, boff:boff + HB]
                nc.vector.tensor_mul(out=hdst,
                                     in0=act[:, 8 * HB:12 * HB], in1=th[:, :])
        # writebacks: own (time-aligned) + reversed copy for the exchange;
        # issued on the scalar queue (SP queue stays free for gx loads)
        hsrc = hb[:, :].rearrange("p (u k b) -> k p u b", k=4, b=BS)
        own = own_chunks[blk // BPC].rearrange("(k p) (s b) -> k p s b",
                                               p=128, b=BS)
        rc = NB - 1 - blk
        rev = rev_chunks[rc // BPC].rearrange("(k p) (s b) -> k p s b",
                                              p=128, b=BS)
        for k in range(4):
            nc.scalar.dma_start(out=own[k, :, ts(lblk, BLK), :],
                                in_=hsrc[k, :, :, :])
            dsl = rev[k, :, ts(rc % BPC, BLK), :]
            rdst = bass.AP(dsl.tensor, dsl.offset + (BLK - 1) * BS,
                           [[CW, 128], [-BS, BLK], [1, BS]])
            nc.scalar.dma_start(out=rdst, in_=hsrc[k, :, :, :])
        prev_hb = hb
        gxt = gxt_next


def _build_full(repeat=1, sim_single_core=False):
    """repeat>1 loops the whole body (slope-based HW timing: wall ~=
    dispatch + repeat*hw). sim_single_core replaces the AllGathers with
    local DRAM copies so TimelineSim (single-core, no collectives) can
    profile the program."""
    nc = bacc.Bacc("TRN2", target_bir_lowering=False, debug=False, num_devices=8)
    xsT = nc.dram_tensor("xsT", [640, NC], BF16, kind="ExternalInput")
    wx1 = nc.dram_tensor("wx1", [640, 2048], BF16, kind="ExternalInput")
    wh1 = nc.dram_tensor("wh1", [512, 2048], BF16, kind="ExternalInput")
    wx2 = nc.dram_tensor("wx2", [1664, 2048], BF16, kind="ExternalInput")
    wh2 = nc.dram_tensor("wh2", [512, 2048], BF16, kind="ExternalInput")
    wtag = nc.dram_tensor("wtag", [1664, 64], BF16, kind="ExternalInput")
    identd = nc.dram_tensor("ident", [128, 128], BF16, kind="ExternalInput")
    tags = nc.dram_tensor("tags", [64, NC], F32, kind="ExternalOutput")

    def allgather(srcs, dsts):
        # dsts[c] is rows 512:1536 of the [1664, CW] input chunk
        for c in range(CH - 1, -1, -1):
            if sim_single_core:
                nc.sync.dma_start(out=dsts[c][0:512, :], in_=srcs[c][:, :])
                nc.sync.dma_start(out=dsts[c][512:1024, :], in_=srcs[c][:, :])
            else:
                nc.gpsimd.collective_compute(
                    "AllGather", mybir.AluOpType.bypass,
                    replica_groups=REPLICA_GROUPS,
                    ins=[srcs[c][:, :]], outs=[dsts[c][:, :]],
                )

    with TileContext(nc) as tc:
        with (
            tc.tile_pool(name="dram", bufs=1, space="DRAM") as dram,
            tc.tile_pool(name="cons", bufs=1) as cons,
            tc.tile_pool(name="wpool", bufs=1) as wpool,
            tc.tile_pool(name="gxpool", bufs=2) as gxpool,
            tc.tile_pool(name="state", bufs=1) as state,
            tc.tile_pool(name="work", bufs=2) as work,
            tc.tile_pool(name="hring", bufs=2) as hring,
            tc.tile_pool(name="psum", bufs=2, space="PSUM") as psum,
            tc.tile_pool(name="psumB", bufs=2 if NH == 1 else 1,
                         space="PSUM") as psumB,
        ):
          pools = (wpool, gxpool, state, work, hring, (psum, psumB))

          def dtile(shape, tag):
              return dram.tile(shape, BF16, tag=tag, name=tag)

          ident = cons.tile([128, 128], BF16, tag="ident")
          nc.sync.dma_start(out=ident[:, :], in_=identd[:, :])

          gx1 = [dtile([2048, CW], f"gx1_{c}") for c in range(CH)]
          gx2 = [dtile([2048, CW], f"gx2_{c}") for c in range(CH)]
          g2in = [dtile([1664, CW], f"g2in_{c}") for c in range(CH)]
          tgin = [dtile([1664, CW], f"tgin_{c}") for c in range(CH)]
          h1r = [dtile([512, CW], f"h1r_{c}") for c in range(CH)]
          h2r = [dtile([512, CW], f"h2r_{c}") for c in range(CH)]

          def gemm(kxm, kxn, out):
              matmul_tile_kernel(tc, kxm, kxn, out,
                                 MAX_TILE_SIZE=512 if NH == 1 else 256)
              tc.swap_default_side()  # undo the kernel's side toggle

          for _rep in range(repeat):
            if _rep:
                # Serialize iterations for slope timing: stage a casted copy
                # of the previous iteration's output into the gx1 chunk the
                # first GEMM overwrites, so iteration r+1 cannot start early.
                ser = cons.tile([64, CW], F32, tag="ser")
                serb = cons.tile([64, CW], BF16, tag="serb")
                nc.sync.dma_start(out=ser[:, :], in_=tags[:, 0:CW])
                nc.vector.tensor_copy(out=serb[:, :], in_=ser[:, :])
                nc.sync.dma_start(out=gx1[0][0:64, :], in_=serb[:, :])

            ones = cons.tile([128, CW], BF16, tag="aug")
            nc.vector.memset(ones[:, :], 0.0)
            nc.vector.memset(ones[0:1, :], 1.0)
            for c in range(CH):
                nc.sync.dma_start(out=g2in[c][1536:1664, :], in_=ones[:, :])
                nc.sync.dma_start(out=tgin[c][1536:1664, :], in_=ones[:, :])

            for c in range(CH):
                gemm(wx1[:, :], xsT[:, ts(c, CW)], gx1[c][:, :])
            _scan_cell(nc, tc, pools, wh1, gx1,
                       [t[0:512, :] for t in g2in], h1r, ident)
            allgather(h1r, [t[512:1536, :] for t in g2in])

            for c in range(CH):
                gemm(wx2[:, :], g2in[c][:, :], gx2[c][:, :])
            _scan_cell(nc, tc, pools, wh2, gx2,
                       [t[0:512, :] for t in tgin], h2r, ident)
            allgather(h2r, [t[512:1536, :] for t in tgin])

            for c in range(CH):
                gemm(wtag[:, :], tgin[c][:, :], tags[:, ts(c, CW)])
    nc.compile()
    return nc


def _bf(x):
    return np.ascontiguousarray(x).astype(ml_dtypes.bfloat16)


def _pack_inputs(words, emb, Wf1, bf1, Wb1, bb1, Wf2, bf2, Wb2, bb2, Wtag, btag):
    words = np.asarray(words).astype(np.int64)
    xs = np.asarray(emb, dtype=np.float32)[words]      # [B, S, E] host gather

    def pack_xs(xs_slice):
        out = np.zeros((640, NC), np.float32)
        out[:512] = xs_slice.transpose(2, 1, 0).reshape(512, NC)
        out[512] = 1.0
        return _bf(out)

    def gates_cat(W, rows):
        return np.concatenate([np.asarray(W[g], dtype=np.float32)[rows]
                               for g in GATE_PERM], axis=1)

    def bias_cat(b):
        return np.concatenate([np.asarray(b[g], dtype=np.float32) for g in GATE_PERM])

    wt = np.asarray(Wtag, dtype=np.float32)
    bt = np.asarray(btag, dtype=np.float32)

    def pack_dir(Wl1, bl1, Wl2, bl2, fwd):
        wx1p = np.zeros((640, 2048), np.float32)
        wx1p[:512] = gates_cat(Wl1, slice(0, 512))
        wx1p[512] = bias_cat(bl1)
        wh1p = gates_cat(Wl1, slice(512, 1024))
        w2x = gates_cat(Wl2, slice(0, 1024))   # rows 0:512 ~ h_f, 512:1024 ~ h_b
        own = slice(0, 512) if fwd else slice(512, 1024)
        peer = slice(512, 1024) if fwd else slice(0, 512)
        # allgathered rows: [0:512] = fwd contribution, [512:1024] = bwd
        peer_off = 1024 if fwd else 512
        wx2p = np.zeros((1664, 2048), np.float32)
        wx2p[0:512] = w2x[own]
        wx2p[peer_off:peer_off + 512] = w2x[peer]
        wx2p[1536] = bias_cat(bl2)
        wh2p = gates_cat(Wl2, slice(1024, 1536))
        wtagp = np.zeros((1664, 64), np.float32)
        wtagp[0:512] = wt[own]
        wtagp[peer_off:peer_off + 512] = wt[peer]
        wtagp[1536] = bt
        return dict(
            wx1=_bf(wx1p), wh1=_bf(wh1p),
            wx2=_bf(wx2p), wh2=_bf(wh2p),
            wtag=_bf(wtagp),
        )

    fw = pack_dir(Wf1, bf1, Wf2, bf2, True)
    bw = pack_dir(Wb1, bb1, Wb2, bb2, False)
    ins = []
    for core in range(8):
        sl = xs[ts(core % 4, BS)] if BS * 4 <= B else xs
        d = dict(fw if core < 4 else bw)
        d["xsT"] = pack_xs(sl if core < 4 else sl[:, ::-1])
        d["ident"] = np.eye(128, dtype=ml_dtypes.bfloat16)
        ins.append(d)
    return ins


_NC_CACHE = {}


def _get_nc():
    if "nc" not in _NC_CACHE:
        _NC_CACHE["nc"] = _build_full()
    return _NC_CACHE["nc"]


def kernel(words, lengths, emb, Wf1, bf1, Wb1, bb1, Wf2, bf2, Wb2, bb2, Wtag, btag):
    nc = _get_nc()
    ins = _pack_inputs(words, emb, Wf1, bf1, Wb1, bb1, Wf2, bf2, Wb2, bb2,
                       Wtag, btag)
    res = run_bass_kernel_spmd(nc, ins, core_ids=list(range(8)))
    if BS * 4 <= B:
        # forward core c holds tags for batch slice [c*BS, (c+1)*BS)
        full = np.concatenate(
            [res.results[c]["tags"].reshape(T, S, BS) for c in range(4)],
            axis=2)
    else:
        full = res.results[0]["tags"].reshape(T, S, B)
    out = full.transpose(2, 1, 0).reshape(B * S, T)
    return np.ascontiguousarray(out.astype(np.float32))


# revision 32
# speedup vs baseline: 2120.3639x; 1.0571x over previous
"""BiLSTM tagger (B=32, S=256, E=H=512, V=50000, T=64) on 8 Trainium2 cores.

Strategy (single SPMD launch, 8-way real parallelism):
  - Core c in 0-3 runs the FORWARD direction for batch slice [8c, 8c+8);
    core c+4 runs the BACKWARD direction for the same slice (its xsT is
    host-reversed in time). All per-core tensors hold 8 batch elements ×
    256 steps = 2048 columns (col = t*8 + b).
  - Per layer: the x-projection (+bias via an augmented ones-row) is a
    small bf16 GEMM; the sequential LSTM recurrence is a 256-step scan
    with weights stationary on the PE (batch=8 moving) and a short
    elementwise chain (in-place psum adds, ch-gates first so the tanh
    path hides under the f/i/o matmuls, fused [f|i]*[c|ch] product, h
    consumed directly from the h ring).
  - Each scan writes its h sequence twice: time-aligned into the next
    GEMM's input buffer, and time-REVERSED into an exchange tensor. The
    pairwise AllGather (core c <-> c+4, same batch slice) of the reversed
    copy lands peer h already aligned to the local time axis, so layer 2
    and the tag projection are each a single GEMM over
    [own-h | allgathered | ones] with no reversed second stream.
  - The host assembles the full [B*S, T] output from the four forward
    cores' batch slices.

Numerics: matmuls bf16 with fp32 PSUM accumulation; gate activations and
cell state fp32; gx streams and h stored bf16.
"""

import os
import numpy as np
import ml_dtypes
import concourse.bass as bass
import concourse.bacc as bacc
import concourse.mybir as mybir
from concourse.tile import TileContext
from concourse.kernels.tile_matmul import matmul_tile_kernel
from concourse.bass_utils import run_bass_kernel_spmd

F32 = mybir.dt.float32
BF16 = mybir.dt.bfloat16
AF = mybir.ActivationFunctionType

S, B, E, H, V, T = 256, 32, 512, 512, 50000, 64
BS = int(os.environ.get("K_BS", "32"))   # batch elements per core
NC = S * BS             # 2048 columns per core
BLK = int(os.environ.get("K_BLK", "8"))  # steps per block
NB = S // BLK           # 32 blocks
CH = 4                  # column chunks (tensors) for gx/exchange buffers
CW = NC // CH           # 512 cols per chunk
BPC = NB // CH          # 8 blocks per chunk
GRP = BLK * BS          # 64 cols per block
REPLICA_GROUPS = [[0, 4], [1, 5], [2, 6], [3, 7]]
GATE_PERM = [0, 1, 3, 2]  # reference gate order [f,i,c,o] -> ours [f,i,o,ch]

ts = lambda i, n: slice(i * n, (i + 1) * n)


NH = int(os.environ.get("K_NH", "1"))   # pipeline sub-streams (batch halves)
HB = BS // NH                           # batch per sub-stream
INJ = int(os.environ.get("K_INJ", "0"))  # 1: inject gx via identity matmul;
                                         # 0: DVE adds into psum


def _scan_cell(nc, tc, pools, wh_in, gx_chunks, own_chunks, rev_chunks, ident):
    """LSTM scan, two interleaved batch sub-streams so one stream's
    elementwise chain overlaps the other's matmuls. Per (step, stream):
    psB [128, 4*HB] holds ch gates (emitted first; its tanh hides under the
    f/i/o matmuls into psA [128, 12*HB]); the x-projection gx is INJECTED
    into psum with an identity matmul (no DVE adds on the chain). gx_chunks:
    CH dram tensors [2048, CW] bf16, consumed time-aligned; own/rev_chunks
    receive h time-aligned / time-reversed."""
    wpool, gxpool, state, work, hring, (psum, psumB) = pools
    wt = []
    for k in range(4):
        w = wpool.tile([128, 2048], BF16, tag=f"wt{k}")
        nc.sync.dma_start(out=w[:, :], in_=wh_in[128 * k:128 * (k + 1), :])
        wt.append(w)
    CB = BS * 4       # one h slot width
    cch = [state.tile([128, 8 * HB], F32, tag=f"cch{g}", name=f"cch{g}")
           for g in range(NH)]          # [c | ch] per stream
    hz = state.tile([128, CB], BF16, tag="hz")
    for g in range(NH):
        nc.vector.memset(cch[g][:, :], 0.0)
    nc.vector.memset(hz[:, :], 0.0)

    def fetch(blk):
        # gxt cols = (m', u, g, b) with m' = [12..15, 0..11]: ch gates first
        gsrc = gx_chunks[blk // BPC].rearrange("(m p) (s b) -> m p s b",
                                               p=128, b=BS)
        gxt = gxpool.tile([128, BLK * 16 * BS], BF16, tag="gx")
        dst = gxt[:, :].rearrange("p (m u b) -> p m u b", m=16, b=BS)
        sl = gsrc[:, :, ts(blk % BPC, BLK), :]
        nc.sync.dma_start(out=dst[:, 0:4],
                          in_=sl[12:16].rearrange("m p u b -> p m u b"))
        nc.sync.dma_start(out=dst[:, 4:16],
                          in_=sl[0:12].rearrange("m p u b -> p m u b"))
        return gxt

    prev_hb = None
    gxt = fetch(0)
    for blk in range(NB):
        lblk = blk % BPC
        gxt_next = fetch(blk + 1) if blk + 1 < NB else None
        hb = hring.tile([128, BLK * CB], BF16, tag="hb")
        for u in range(BLK):
            if u == 0:
                hsrc_prev = hz if prev_hb is None else prev_hb
                hoff = 0 if prev_hb is None else (BLK - 1) * CB
            else:
                hsrc_prev = hb
                hoff = (u - 1) * CB
            g4 = gxt[:, :].rearrange("p (m u g b) -> p m u g b",
                                     m=16, u=BLK, g=NH)
            for g in range(NH):
                boff = HB * g
                psA = psum.tile([128, 12 * HB], F32, tag=f"psA{g}",
                                name=f"psA{g}")
                psB = psumB.tile([128, 4 * HB], F32, tag=f"psB{g}",
                                 name=f"psB{g}")
                for m in range(12, 16):
                    for k in range(4):
                        nc.tensor.matmul(
                            psB[:, ts(m - 12, HB)],
                            lhsT=wt[k][:, 128 * m:128 * (m + 1)],
                            rhs=hsrc_prev[:, hoff + BS * k + boff:
                                          hoff + BS * k + boff + HB],
                            start=(k == 0), stop=(not INJ and k == 3),
                        )
                if INJ:
                    nc.tensor.matmul(psB[:, :], lhsT=ident[:, :],
                                     rhs=g4[:, 0:4, u, g, :],
                                     start=False, stop=True)
                else:
                    nc.vector.tensor_add(
                        out=psB[:, :].rearrange("p (m b) -> p m b", b=HB),
                        in0=psB[:, :].rearrange("p (m b) -> p m b", b=HB),
                        in1=g4[:, 0:4, u, g, :])
                for m in range(12):
                    for k in range(4):
                        nc.tensor.matmul(
                            psA[:, ts(m, HB)],
                            lhsT=wt[k][:, 128 * m:128 * (m + 1)],
                            rhs=hsrc_prev[:, hoff + BS * k + boff:
                                          hoff + BS * k + boff + HB],
                            start=(k == 0), stop=(not INJ and k == 3),
                        )
                if INJ:
                    nc.tensor.matmul(psA[:, :], lhsT=ident[:, :],
                                     rhs=g4[:, 4:16, u, g, :],
                                     start=False, stop=True)
                else:
                    nc.vector.tensor_add(
                        out=psA[:, :].rearrange("p (m b) -> p m b", b=HB),
                        in0=psA[:, :].rearrange("p (m b) -> p m b", b=HB),
                        in1=g4[:, 4:16, u, g, :])
                nc.scalar.activation(cch[g][:, 4 * HB:8 * HB], psB[:, :],
                                     AF.Tanh)
                act = work.tile([128, 12 * HB], F32, tag=f"act{g}",
                                name=f"act{g}")  # [f|i|o]
                nc.scalar.activation(act[:, :], psA[:, :], AF.Sigmoid)
                P = work.tile([128, 8 * HB], F32, tag=f"P{g}", name=f"P{g}")
                nc.vector.tensor_mul(out=P[:, :], in0=act[:, 0:8 * HB],
                                     in1=cch[g][:, :])
                nc.vector.tensor_add(out=cch[g][:, 0:4 * HB], in0=P[:, 0:4 * HB],
                                     in1=P[:, 4 * HB:8 * HB])
                th = work.tile([128, 4 * HB], F32, tag=f"th{g}", name=f"th{g}")
                nc.scalar.activation(th[:, :], cch[g][:, 0:4 * HB], AF.Tanh)
                hdst = hb[:, :].rearrange("p (uu k b) -> p uu k b",
                                          uu=BLK, b=BS)[:, u, :, boff:boff + HB]
                nc.vector.tensor_mul(out=hdst,
                                     in0=act[:, 8 * HB:12 * HB], in1=th[:, :])
        # writebacks: own (time-aligned) + reversed copy for the exchange;
        # issued on the scalar queue (SP queue stays free for gx loads)
        hsrc = hb[:, :].rearrange("p (u k b) -> k p u b", k=4, b=BS)
        own = own_chunks[blk // BPC].rearrange("(k p) (s b) -> k p s b",
                                               p=128, b=BS)
        rc = NB - 1 - blk
        rev = rev_chunks[rc // BPC].rearrange("(k p) (s b) -> k p s b",
                                              p=128, b=BS)
        for k in range(4):
            nc.scalar.dma_start(out=own[k, :, ts(lblk, BLK), :],
                                in_=hsrc[k, :, :, :])
            dsl = rev[k, :, ts(rc % BPC, BLK), :]
            rdst = bass.AP(dsl.tensor, dsl.offset + (BLK - 1) * BS,
                           [[CW, 128], [-BS, BLK], [1, BS]])
            nc.scalar.dma_start(out=rdst, in_=hsrc[k, :, :, :])
        prev_hb = hb
        gxt = gxt_next


def _build_full(repeat=1, sim_single_core=False):
    """repeat>1 loops the whole body (slope-based HW timing: wall ~=
    dispatch + repeat*hw). sim_single_core replaces the AllGathers with
    local DRAM copies so TimelineSim (single-core, no collectives) can
    profile the program."""
    nc = bacc.Bacc("TRN2", target_bir_lowering=False, debug=False, num_devices=8)
    xsT = nc.dram_tensor("xsT", [640, NC], BF16, kind="ExternalInput")
    wx1 = nc.dram_tensor("wx1", [640, 2048], BF16, kind="ExternalInput")
    wh1 = nc.dram_tensor("wh1", [512, 2048], BF16, kind="ExternalInput")
    wx2 = nc.dram_tensor("wx2", [1664, 2048], BF16, kind="ExternalInput")
    wh2 = nc.dram_tensor("wh2", [512, 2048], BF16, kind="ExternalInput")
    wtag = nc.dram_tensor("wtag", [1664, 64], BF16, kind="ExternalInput")
    identd = nc.dram_tensor("ident", [128, 128], BF16, kind="ExternalInput")
    tags = nc.dram_tensor("tags", [64, NC], F32, kind="ExternalOutput")

    def allgather(srcs, dsts):
        # dsts[c] is rows 512:1536 of the [1664, CW] input chunk
        for c in range(CH - 1, -1, -1):
            if sim_single_core:
                nc.sync.dma_start(out=dsts[c][0:512, :], in_=srcs[c][:, :])
                nc.sync.dma_start(out=dsts[c][512:1024, :], in_=srcs[c][:, :])
            else:
                nc.gpsimd.collective_compute(
                    "AllGather", mybir.AluOpType.bypass,
                    replica_groups=REPLICA_GROUPS,
                    ins=[srcs[c][:, :]], outs=[dsts[c][:, :]],
                )

    with TileContext(nc) as tc:
        with (
            tc.tile_pool(name="dram", bufs=1, space="DRAM") as dram,
            tc.tile_pool(name="cons", bufs=1) as cons,
            tc.tile_pool(name="wpool", bufs=1) as wpool,
            tc.tile_pool(name="gxpool", bufs=int(os.environ.get("K_GXB", "2"))) as gxpool,
            tc.tile_pool(name="state", bufs=1) as state,
            tc.tile_pool(name="work", bufs=int(os.environ.get("K_WB", "2"))) as work,
            tc.tile_pool(name="hring", bufs=2) as hring,
            tc.tile_pool(name="psum", bufs=2, space="PSUM") as psum,
            tc.tile_pool(name="psumB", bufs=2 if NH == 1 else 1,
                         space="PSUM") as psumB,
        ):
          pools = (wpool, gxpool, state, work, hring, (psum, psumB))

          def dtile(shape, tag):
              return dram.tile(shape, BF16, tag=tag, name=tag)

          ident = cons.tile([128, 128], BF16, tag="ident")
          nc.sync.dma_start(out=ident[:, :], in_=identd[:, :])

          gx1 = [dtile([2048, CW], f"gx1_{c}") for c in range(CH)]
          gx2 = [dtile([2048, CW], f"gx2_{c}") for c in range(CH)]
          g2in = [dtile([1664, CW], f"g2in_{c}") for c in range(CH)]
          tgin = [dtile([1664, CW], f"tgin_{c}") for c in range(CH)]
          h1r = [dtile([512, CW], f"h1r_{c}") for c in range(CH)]
          h2r = [dtile([512, CW], f"h2r_{c}") for c in range(CH)]

          def gemm(kxm, kxn, out):
              matmul_tile_kernel(tc, kxm, kxn, out,
                                 MAX_TILE_SIZE=512 if NH == 1 else 256)
              tc.swap_default_side()  # undo the kernel's side toggle

          for _rep in range(repeat):
            if _rep:
                # Serialize iterations for slope timing: stage a casted copy
                # of the previous iteration's output into the gx1 chunk the
                # first GEMM overwrites, so iteration r+1 cannot start early.
                ser = cons.tile([64, CW], F32, tag="ser")
                serb = cons.tile([64, CW], BF16, tag="serb")
                nc.sync.dma_start(out=ser[:, :], in_=tags[:, 0:CW])
                nc.vector.tensor_copy(out=serb[:, :], in_=ser[:, :])
                nc.sync.dma_start(out=gx1[0][0:64, :], in_=serb[:, :])

            ones = cons.tile([128, CW], BF16, tag="aug")
            nc.vector.memset(ones[:, :], 0.0)
            nc.vector.memset(ones[0:1, :], 1.0)
            for c in range(CH):
                nc.sync.dma_start(out=g2in[c][1536:1664, :], in_=ones[:, :])
                nc.sync.dma_start(out=tgin[c][1536:1664, :], in_=ones[:, :])

            for c in range(CH):
                gemm(wx1[:, :], xsT[:, ts(c, CW)], gx1[c][:, :])
            _scan_cell(nc, tc, pools, wh1, gx1,
                       [t[0:512, :] for t in g2in], h1r, ident)
            allgather(h1r, [t[512:1536, :] for t in g2in])

            for c in range(CH):
                gemm(wx2[:, :], g2in[c][:, :], gx2[c][:, :])
            _scan_cell(nc, tc, pools, wh2, gx2,
                       [t[0:512, :] for t in tgin], h2r, ident)
            allgather(h2r, [t[512:1536, :] for t in tgin])

            for c in range(CH):
                gemm(wtag[:, :], tgin[c][:, :], tags[:, ts(c, CW)])
    nc.compile()
    return nc


def _bf(x):
    return np.ascontiguousarray(x).astype(ml_dtypes.bfloat16)


def _pack_inputs(words, emb, Wf1, bf1, Wb1, bb1, Wf2, bf2, Wb2, bb2, Wtag, btag):
    words = np.asarray(words).astype(np.int64)
    xs = np.asarray(emb, dtype=np.float32)[words]      # [B, S, E] host gather

    def pack_xs(xs_slice):
        out = np.zeros((640, NC), np.float32)
        out[:512] = xs_slice.transpose(2, 1, 0).reshape(512, NC)
        out[512] = 1.0
        return _bf(out)

    def gates_cat(W, rows):
        return np.concatenate([np.asarray(W[g], dtype=np.float32)[rows]
                               for g in GATE_PERM], axis=1)

    def bias_cat(b):
        return np.concatenate([np.asarray(b[g], dtype=np.float32) for g in GATE_PERM])

    wt = np.asarray(Wtag, dtype=np.float32)
    bt = np.asarray(btag, dtype=np.float32)

    def pack_dir(Wl1, bl1, Wl2, bl2, fwd):
        wx1p = np.zeros((640, 2048), np.float32)
        wx1p[:512] = gates_cat(Wl1, slice(0, 512))
        wx1p[512] = bias_cat(bl1)
        wh1p = gates_cat(Wl1, slice(512, 1024))
        w2x = gates_cat(Wl2, slice(0, 1024))   # rows 0:512 ~ h_f, 512:1024 ~ h_b
        own = slice(0, 512) if fwd else slice(512, 1024)
        peer = slice(512, 1024) if fwd else slice(0, 512)
        # allgathered rows: [0:512] = fwd contribution, [512:1024] = bwd
        peer_off = 1024 if fwd else 512
        wx2p = np.zeros((1664, 2048), np.float32)
        wx2p[0:512] = w2x[own]
        wx2p[peer_off:peer_off + 512] = w2x[peer]
        wx2p[1536] = bias_cat(bl2)
        wh2p = gates_cat(Wl2, slice(1024, 1536))
        wtagp = np.zeros((1664, 64), np.float32)
        wtagp[0:512] = wt[own]
        wtagp[peer_off:peer_off + 512] = wt[peer]
        wtagp[1536] = bt
        return dict(
            wx1=_bf(wx1p), wh1=_bf(wh1p),
            wx2=_bf(wx2p), wh2=_bf(wh2p),
            wtag=_bf(wtagp),
        )

    fw = pack_dir(Wf1, bf1, Wf2, bf2, True)
    bw = pack_dir(Wb1, bb1, Wb2, bb2, False)
    ins = []
    for core in range(8):
        sl = xs[ts(core % 4, BS)] if BS * 4 <= B else xs
        d = dict(fw if core < 4 else bw)
        d["xsT"] = pack_xs(sl if core < 4 else sl[:, ::-1])
        d["ident"] = np.eye(128, dtype=ml_dtypes.bfloat16)
        ins.append(d)
    return ins


_NC_CACHE = {}


def _get_nc():
    if "nc" not in _NC_CACHE:
        _NC_CACHE["nc"] = _build_full()
    return _NC_CACHE["nc"]


def kernel(words, lengths, emb, Wf1, bf1, Wb1, bb1, Wf2, bf2, Wb2, bb2, Wtag, btag):
    nc = _get_nc()
    ins = _pack_inputs(words, emb, Wf1, bf1, Wb1, bb1, Wf2, bf2, Wb2, bb2,
                       Wtag, btag)
    res = run_bass_kernel_spmd(nc, ins, core_ids=list(range(8)))
    if BS * 4 <= B:
        # forward core c holds tags for batch slice [c*BS, (c+1)*BS)
        full = np.concatenate(
            [res.results[c]["tags"].reshape(T, S, BS) for c in range(4)],
            axis=2)
    else:
        full = res.results[0]["tags"].reshape(T, S, B)
    out = full.transpose(2, 1, 0).reshape(B * S, T)
    return np.ascontiguousarray(out.astype(np.float32))
